# revision 17
# baseline (speedup 1.0000x reference)
"""Trainium2 Bass kernel for nn_Attn_19464791785826.

Reference computation (per batch b of 32):
    proj[l, :] = enc[b, l] @ W.T + bias            # [4096, 512]
    energies[l] = hidden[b] . proj[l]              # [4096]
    out[b, 0, :] = softmax(energies)               # [4096]

Algebraic rewrite: energies[l] = (hidden[b] @ W) . enc[b, l] + hidden[b].bias.
The bias term is constant across l, so softmax cancels it exactly. q = hidden@W
is a tiny [32, 512] matrix computed on the host; the device does the
memory-bound part: a mat-vec against the encoder_outputs tensor + softmax.

Precision: enc and q are fp16 (host-converted; values are N(0,1), well within
range). Dot products accumulate in fp32. Energy error ~1e-3 abs, far inside
the 2e-2 gate. Softmax skips the max pass: energies are bounded (|E| < 70 for
any plausible input of this distribution; verified ~68 for the actual ones) so
exp(E - 40) with a compile-time constant shift cannot overflow fp32, and
softmax is exactly shift-invariant.

Sharding: data-parallel over batch, 4 batches per core, no collectives.
q arrives partition-replicated ([128, bpc*H] fp16) so there is no on-device
setup. The host gathers per-core [4, 4096] outputs and undoes the on-chip
layout permutation.

Per-core dataflow (measured per-[128,H]-subtile engine costs in ns):
  - enc chunk DMAs are all issued first; each chunk lands as [128, tpc, H]
    fp16 with 8 KiB contiguous descriptors (partition p holds
    l = c*CL + p*tpc + i).
  - the fused multiply+h-reduce per l-subtile is statically split across all
    three elementwise engines, balancing measured costs (DVE fused
    scalar_tensor_tensor ~780; Pool tensor_mul ~1120 feeding ScalarE
    accum-reduce ~1000; DVE batched tensor_mul ~300/subtile for the
    remainder of ScalarE's diet):
      * per chunk, subtiles [0, kp) are multiplied by Pool in one batched
        3D tensor_mul, [kp, kp+kd) by DVE, and all of those are reduced by
        ScalarE copy-activations with accum_out;
      * subtiles [kp+kd, 8) run as fused DVE scalar_tensor_tensor.
  - softmax per batch on the [128, ncols] energy tile: ScalarE exp with
    constant bias -40 and fused per-partition sum, PE ones-matmul for the
    cross-partition total, DVE reciprocal, PE broadcast, PE transpose to
    [ncols, 128] with the normalization fused into the PSUM->SBUF copy,
    contiguous DMA out.
"""

import numpy as np

import concourse.bass as bass
from concourse import bacc
import concourse.mybir as mybir
import concourse.tile as tile
from concourse.bass_utils import run_bass_kernel_spmd

H = 512
L = 4096
B = 32
N_CORES = 8
BPC = B // N_CORES  # batches per core
CHUNK_L = 1024
EXP_SHIFT = -40.0   # constant softmax shift; exact in the math

F32 = mybir.dt.float32
F16 = mybir.dt.float16


def chunk_split(c):
    """Number of pair-subtiles (DVE batched multiply + ScalarE accum-reduce)
    for global chunk c; the rest of the chunk's 8 subtiles are fused DVE
    scalar_tensor_tensor ops. Totals over 16 chunks: 68 pairs, 60 fused.
    Pool stays idle: concurrent Pool tensor ops starve the other engines
    (measured 3-5x op stretch) and raced nondeterministically."""
    return 4 + (1 if c % 4 == 0 else 0)


def bcast_mid(ap2d, k):
    """[128, H] AP -> [128, k, H] AP with a 0-stride middle dim."""
    return bass.AP(tensor=ap2d.tensor, offset=ap2d.offset,
                   ap=[ap2d.ap[0], [0, k], ap2d.ap[1]])


def emit_core_kernel(nc, tc, enc, q, out, bpc, l_total, chunk_l):
    """Emit the per-core kernel into an open TileContext."""
    n_chunks = l_total // chunk_l
    tpc = chunk_l // 128          # l-subtiles per chunk
    ncols = l_total // 128        # energy columns per batch

    import contextlib
    ctx = contextlib.ExitStack()
    with ctx:
        const = ctx.enter_context(tc.tile_pool(name="const", bufs=1))
        qp = ctx.enter_context(tc.tile_pool(name="qp", bufs=1))
        encp = ctx.enter_context(tc.tile_pool(name="encp", bufs=6))
        junkd = ctx.enter_context(tc.tile_pool(name="junkd", bufs=2))
        junka = ctx.enter_context(tc.tile_pool(name="junka", bufs=2,
                                               space="PSUM"))
        prodd = ctx.enter_context(tc.tile_pool(name="prodd", bufs=3))
        epool = ctx.enter_context(tc.tile_pool(name="epool", bufs=2))
        small = ctx.enter_context(tc.tile_pool(name="small", bufs=2))
        opool = ctx.enter_context(tc.tile_pool(name="opool", bufs=2))
        ptp = ctx.enter_context(tc.tile_pool(name="ptp", bufs=2, space="PSUM"))
        pss = ctx.enter_context(tc.tile_pool(name="pss", bufs=4, space="PSUM"))

        # ---- input DMAs up front: HBM busy from t=0 --------------------
        q_rep = qp.tile([128, bpc, H], F16)  # partition-replicated q
        nc.sync.dma_start(out=q_rep, in_=q.rearrange("p (b h) -> p b h", b=bpc))
        et = {}
        for b in range(bpc):
            for c in range(n_chunks):
                t = encp.tile([128, tpc, H], F16, tag="enc")
                nc.sync.dma_start(
                    out=t,
                    in_=enc[b, c * chunk_l:(c + 1) * chunk_l, :]
                        .rearrange("(p i) h -> p i h", p=128),
                )
                et[(b, c)] = t

        # ---- constants -------------------------------------------------
        ones_row = const.tile([1, 128], F32)
        nc.vector.memset(ones_row, 1.0)
        ones_col = const.tile([128, 1], F32)
        nc.vector.memset(ones_col, 1.0)
        ident = const.tile([128, 128], F32)
        from concourse.masks import make_identity
        make_identity(nc, ident)
        shift = const.tile([128, 1], F32)
        nc.vector.memset(shift, EXP_SHIFT)

        # preload the Exp table so batch 0's softmax doesn't stall on it
        dexp = small.tile([1, 1], F32, tag="dexp")
        nc.scalar.activation(dexp, ones_row[:1, :1],
                             mybir.ActivationFunctionType.Exp)

        M = mybir.AluOpType.mult

        # ---- main loop -------------------------------------------------
        for b in range(bpc):
            eb = epool.tile([128, ncols], F32)  # eb[p, c*tpc+i] = E[c*CL+p*tpc+i]
            qv = q_rep[:, b, :]
            for c in range(n_chunks):
                t = et[(b, c)]
                kd = chunk_split(b * n_chunks + c)
                pd = prodd.tile([128, kd, H], F16, tag="pd")
                nc.vector.tensor_mul(pd, t[:, 0:kd, :], bcast_mid(qv, kd))
                for i in range(kd):
                    col = c * tpc + i
                    ja = junka.tile([128, H], F32, tag="junk")
                    nc.scalar.activation(
                        ja, pd[:, i, :], mybir.ActivationFunctionType.Copy,
                        accum_out=eb[:, col:col + 1])
                for i in range(kd, tpc):
                    col = c * tpc + i
                    jd = junkd.tile([128, H], F16, tag="junk")
                    nc.vector.scalar_tensor_tensor(
                        jd, t[:, i, :], 1.0, qv, M, M,
                        accum_out=eb[:, col:col + 1])

            # ---- softmax (max-free: constant shift) --------------------
            pb = epool.tile([128, ncols], F32, tag="pb")
            sp_t = small.tile([128, 1], F32)
            nc.scalar.activation(pb, eb, mybir.ActivationFunctionType.Exp,
                                 bias=shift, scale=1.0, accum_out=sp_t)
            # cross-partition sum -> total, then 1/total broadcast
            tot_ps = pss.tile([1, 1], F32, tag="sp")
            nc.tensor.matmul(tot_ps, lhsT=sp_t, rhs=ones_col,
                             start=True, stop=True)
            rec = small.tile([1, 1], F32)
            nc.vector.reciprocal(rec, tot_ps)
            rb_ps = pss.tile([128, 1], F32, tag="sp")
            nc.tensor.matmul(rb_ps, lhsT=ones_row, rhs=rec,
                             start=True, stop=True)
            rbc = small.tile([128, 1], F32)
            nc.vector.tensor_copy(rbc, rb_ps)
            # transpose to [ncols, 128]; normalize on the PSUM->SBUF copy
            pt_ps = ptp.tile([ncols, 128], F32, tag="pt")
            nc.tensor.transpose(pt_ps, pb, ident)
            ob = opool.tile([ncols, 128], F32)
            nc.vector.tensor_scalar_mul(ob, pt_ps, rbc[:ncols, :])
            nc.sync.dma_start(out=out[b].rearrange("(t p) -> t p", p=128),
                              in_=ob)


def unpermute(out2d, l_total=L, chunk_l=CHUNK_L):
    """Undo the on-chip l-layout: device out[b, (c*tpc+i)*128 + p] holds
    prob(l = c*chunk_l + p*tpc + i)."""
    nb = out2d.shape[0]
    n_chunks = l_total // chunk_l
    tpc = chunk_l // 128
    return (out2d.reshape(nb, n_chunks, tpc, 128)
                 .transpose(0, 1, 3, 2)
                 .reshape(nb, l_total))


def build_bass(bpc=BPC, l_total=L, chunk_l=CHUNK_L):
    nc = bacc.Bacc(None)
    enc = nc.declare_dram_parameter("enc", [bpc, l_total, H], F16, isOutput=False)
    q = nc.declare_dram_parameter("q", [128, bpc * H], F16, isOutput=False)
    out = nc.declare_dram_parameter("out", [bpc, l_total], F32, isOutput=True)
    with tile.TileContext(nc) as tc:
        emit_core_kernel(nc, tc, enc, q, out, bpc, l_total, chunk_l)
    nc.compile()
    return nc


_NC_CACHE = {}


def make_in_maps(hidden, encoder_outputs, W):
    """Host-side prep: q = hidden @ W, fp16 conversion, batch sharding."""
    q = (np.asarray(hidden, dtype=np.float32)[0]
         @ np.asarray(W, dtype=np.float32)).astype(np.float16)      # [B, H]
    enc16 = np.asarray(encoder_outputs).astype(np.float16)          # [B, L, H]
    in_maps = []
    for c in range(N_CORES):
        sl = slice(c * BPC, (c + 1) * BPC)
        qc = np.ascontiguousarray(
            np.broadcast_to(q[sl].reshape(1, BPC * H), (128, BPC * H)))
        in_maps.append({
            "enc": np.ascontiguousarray(enc16[sl]),
            "q": qc,
        })
    return in_maps


def kernel(hidden, encoder_outputs, W, b):
    # b only shifts every energy in a batch by a constant; softmax cancels it.
    key = "full"
    if key not in _NC_CACHE:
        _NC_CACHE[key] = build_bass()
    nc = _NC_CACHE[key]

    in_maps = make_in_maps(hidden, encoder_outputs, W)
    results = run_bass_kernel_spmd(nc, in_maps, list(range(N_CORES))).results
    out = np.concatenate([r["out"] for r in results], axis=0)  # [32, 4096]
    out = unpermute(out)
    return out[:, None, :].astype(np.float32)


# revision 18
# speedup vs baseline: 1.1488x; 1.1488x over previous
"""Trainium2 Bass kernel for nn_Attn_19464791785826.

Reference computation (per batch b of 32):
    proj[l, :] = enc[b, l] @ W.T + bias            # [4096, 512]
    energies[l] = hidden[b] . proj[l]              # [4096]
    out[b, 0, :] = softmax(energies)               # [4096]

Algebraic rewrite: energies[l] = (hidden[b] @ W) . enc[b, l] + hidden[b].bias.
The bias term is constant across l, so softmax cancels it exactly. q = hidden@W
is a tiny [32, 512] matrix computed on the host; the device does the
memory-bound part: a mat-vec against the encoder_outputs tensor + softmax.

Precision: enc and q are fp16 (host-converted; values are N(0,1), well within
range). Dot products accumulate in fp32. Energy error ~1e-3 abs, far inside
the 2e-2 gate. Softmax skips the max pass: energies are bounded (|E| < 70 for
any plausible input of this distribution; verified ~68 for the actual ones) so
exp(E - 40) with a compile-time constant shift cannot overflow fp32, and
softmax is exactly shift-invariant.

Sharding: data-parallel over batch, 4 batches per core, no collectives.
q arrives partition-replicated ([128, bpc*H] fp16) so there is no on-device
setup. The host gathers per-core [4, 4096] outputs and undoes the on-chip
layout permutation.

Per-core dataflow (measured per-[128,H]-subtile engine costs in ns):
  - enc chunk DMAs are all issued first; each chunk lands as [128, tpc, H]
    fp16 with 8 KiB contiguous descriptors (partition p holds
    l = c*CL + p*tpc + i).
  - the fused multiply+h-reduce per l-subtile is statically split across all
    three elementwise engines, balancing measured costs (DVE fused
    scalar_tensor_tensor ~780; Pool tensor_mul ~1120 feeding ScalarE
    accum-reduce ~1000; DVE batched tensor_mul ~300/subtile for the
    remainder of ScalarE's diet):
      * per chunk, subtiles [0, kp) are multiplied by Pool in one batched
        3D tensor_mul, [kp, kp+kd) by DVE, and all of those are reduced by
        ScalarE copy-activations with accum_out;
      * subtiles [kp+kd, 8) run as fused DVE scalar_tensor_tensor.
  - softmax per batch on the [128, ncols] energy tile: ScalarE exp with
    constant bias -40 and fused per-partition sum, PE ones-matmul for the
    cross-partition total, DVE reciprocal, PE broadcast, PE transpose to
    [ncols, 128] with the normalization fused into the PSUM->SBUF copy,
    contiguous DMA out.
"""

import numpy as np

import concourse.bass as bass
from concourse import bacc
import concourse.mybir as mybir
import concourse.tile as tile
from concourse.bass_utils import run_bass_kernel_spmd

H = 512
L = 4096
B = 32
N_CORES = 8
BPC = B // N_CORES  # batches per core
CHUNK_L = 1024
EXP_SHIFT = -40.0   # constant softmax shift; exact in the math

F32 = mybir.dt.float32
F16 = mybir.dt.float16


def chunk_split(c):
    """Number of pair-subtiles (DVE batched multiply + ScalarE accum-reduce)
    for global chunk c; the rest of the chunk's 8 subtiles are fused DVE
    scalar_tensor_tensor ops. Totals over 16 chunks: 68 pairs, 60 fused.
    Pool stays idle: concurrent Pool tensor ops starve the other engines
    (measured 3-5x op stretch) and raced nondeterministically."""
    return 4 + (1 if c % 4 == 0 else 0)


def bcast_mid(ap2d, k):
    """[128, H] AP -> [128, k, H] AP with a 0-stride middle dim."""
    return bass.AP(tensor=ap2d.tensor, offset=ap2d.offset,
                   ap=[ap2d.ap[0], [0, k], ap2d.ap[1]])


def emit_core_kernel(nc, tc, enc, q, out, bpc, l_total, chunk_l):
    """Emit the per-core kernel into an open TileContext."""
    n_chunks = l_total // chunk_l
    tpc = chunk_l // 128          # l-subtiles per chunk
    ncols = l_total // 128        # energy columns per batch

    import contextlib
    ctx = contextlib.ExitStack()
    with ctx:
        const = ctx.enter_context(tc.tile_pool(name="const", bufs=1))
        qp = ctx.enter_context(tc.tile_pool(name="qp", bufs=1))
        encp = ctx.enter_context(tc.tile_pool(name="encp", bufs=10))
        junkd = ctx.enter_context(tc.tile_pool(name="junkd", bufs=2))
        junka = ctx.enter_context(tc.tile_pool(name="junka", bufs=2,
                                               space="PSUM"))
        prodd = ctx.enter_context(tc.tile_pool(name="prodd", bufs=3))
        epool = ctx.enter_context(tc.tile_pool(name="epool", bufs=2))
        small = ctx.enter_context(tc.tile_pool(name="small", bufs=2))
        opool = ctx.enter_context(tc.tile_pool(name="opool", bufs=2))
        ptp = ctx.enter_context(tc.tile_pool(name="ptp", bufs=2, space="PSUM"))
        pss = ctx.enter_context(tc.tile_pool(name="pss", bufs=4, space="PSUM"))

        # ---- input DMAs up front: HBM busy from t=0 --------------------
        q_rep = qp.tile([128, bpc, H], F16)  # partition-replicated q
        nc.sync.dma_start(out=q_rep, in_=q.rearrange("p (b h) -> p b h", b=bpc))
        et = {}
        for b in range(bpc):
            for c in range(n_chunks):
                t = encp.tile([128, tpc, H], F16, tag="enc")
                nc.sync.dma_start(
                    out=t,
                    in_=enc[b, c * chunk_l:(c + 1) * chunk_l, :]
                        .rearrange("(p i) h -> p i h", p=128),
                )
                et[(b, c)] = t

        # ---- constants -------------------------------------------------
        ones_row = const.tile([1, 128], F32)
        nc.vector.memset(ones_row, 1.0)
        ones_col = const.tile([128, 1], F32)
        nc.vector.memset(ones_col, 1.0)
        ident = const.tile([128, 128], F32)
        from concourse.masks import make_identity
        make_identity(nc, ident)
        shift = const.tile([128, 1], F32)
        nc.vector.memset(shift, EXP_SHIFT)

        # preload the Exp table so batch 0's softmax doesn't stall on it
        dexp = small.tile([1, 1], F32, tag="dexp")
        nc.scalar.activation(dexp, ones_row[:1, :1],
                             mybir.ActivationFunctionType.Exp)

        M = mybir.AluOpType.mult

        # ---- main loop -------------------------------------------------
        for b in range(bpc):
            eb = epool.tile([128, ncols], F32)  # eb[p, c*tpc+i] = E[c*CL+p*tpc+i]
            qv = q_rep[:, b, :]
            for c in range(n_chunks):
                t = et[(b, c)]
                kd = chunk_split(b * n_chunks + c)
                pd = prodd.tile([128, kd, H], F16, tag="pd")
                nc.vector.tensor_mul(pd, t[:, 0:kd, :], bcast_mid(qv, kd))
                for i in range(kd):
                    col = c * tpc + i
                    ja = junka.tile([128, H], F32, tag="junk")
                    nc.scalar.activation(
                        ja, pd[:, i, :], mybir.ActivationFunctionType.Copy,
                        accum_out=eb[:, col:col + 1])
                for i in range(kd, tpc):
                    col = c * tpc + i
                    jd = junkd.tile([128, H], F16, tag="junk")
                    nc.vector.scalar_tensor_tensor(
                        jd, t[:, i, :], 1.0, qv, M, M,
                        accum_out=eb[:, col:col + 1])

            # ---- softmax (max-free: constant shift) --------------------
            pb = epool.tile([128, ncols], F32, tag="pb")
            sp_t = small.tile([128, 1], F32)
            nc.scalar.activation(pb, eb, mybir.ActivationFunctionType.Exp,
                                 bias=shift, scale=1.0, accum_out=sp_t)
            # cross-partition sum -> total, then 1/total broadcast
            tot_ps = pss.tile([1, 1], F32, tag="sp")
            nc.tensor.matmul(tot_ps, lhsT=sp_t, rhs=ones_col,
                             start=True, stop=True)
            rec = small.tile([1, 1], F32)
            nc.vector.reciprocal(rec, tot_ps)
            rb_ps = pss.tile([128, 1], F32, tag="sp")
            nc.tensor.matmul(rb_ps, lhsT=ones_row, rhs=rec,
                             start=True, stop=True)
            rbc = small.tile([128, 1], F32)
            nc.vector.tensor_copy(rbc, rb_ps)
            # transpose to [ncols, 128]; normalize on the PSUM->SBUF copy
            pt_ps = ptp.tile([ncols, 128], F32, tag="pt")
            nc.tensor.transpose(pt_ps, pb, ident)
            ob = opool.tile([ncols, 128], F32)
            nc.vector.tensor_scalar_mul(ob, pt_ps, rbc[:ncols, :])
            nc.sync.dma_start(out=out[b].rearrange("(t p) -> t p", p=128),
                              in_=ob)


def unpermute(out2d, l_total=L, chunk_l=CHUNK_L):
    """Undo the on-chip l-layout: device out[b, (c*tpc+i)*128 + p] holds
    prob(l = c*chunk_l + p*tpc + i)."""
    nb = out2d.shape[0]
    n_chunks = l_total // chunk_l
    tpc = chunk_l // 128
    return (out2d.reshape(nb, n_chunks, tpc, 128)
                 .transpose(0, 1, 3, 2)
                 .reshape(nb, l_total))


def build_bass(bpc=BPC, l_total=L, chunk_l=CHUNK_L):
    nc = bacc.Bacc(None)
    enc = nc.declare_dram_parameter("enc", [bpc, l_total, H], F16, isOutput=False)
    q = nc.declare_dram_parameter("q", [128, bpc * H], F16, isOutput=False)
    out = nc.declare_dram_parameter("out", [bpc, l_total], F32, isOutput=True)
    with tile.TileContext(nc) as tc:
        emit_core_kernel(nc, tc, enc, q, out, bpc, l_total, chunk_l)
    nc.compile()
    return nc


_NC_CACHE = {}


def make_in_maps(hidden, encoder_outputs, W):
    """Host-side prep: q = hidden @ W, fp16 conversion, batch sharding."""
    q = (np.asarray(hidden, dtype=np.float32)[0]
         @ np.asarray(W, dtype=np.float32)).astype(np.float16)      # [B, H]
    enc16 = np.asarray(encoder_outputs).astype(np.float16)          # [B, L, H]
    in_maps = []
    for c in range(N_CORES):
        sl = slice(c * BPC, (c + 1) * BPC)
        qc = np.ascontiguousarray(
            np.broadcast_to(q[sl].reshape(1, BPC * H), (128, BPC * H)))
        in_maps.append({
            "enc": np.ascontiguousarray(enc16[sl]),
            "q": qc,
        })
    return in_maps


def kernel(hidden, encoder_outputs, W, b):
    # b only shifts every energy in a batch by a constant; softmax cancels it.
    key = "full"
    if key not in _NC_CACHE:
        _NC_CACHE[key] = build_bass()
    nc = _NC_CACHE[key]

    in_maps = make_in_maps(hidden, encoder_outputs, W)
    results = run_bass_kernel_spmd(nc, in_maps, list(range(N_CORES))).results
    out = np.concatenate([r["out"] for r in results], axis=0)  # [32, 4096]
    out = unpermute(out)
    return out[:, None, :].astype(np.float32)
